# revision 25
# baseline (speedup 1.0000x reference)
"""ChannelRoll Trainium2 Bass kernel.

out[b,h,w,c] = x[b,h,w,(c + shift_map[b,h,w,0]) % 256]

Strategy (pure data-parallel over batch, 8 cores):
  - Each core gets 4 batches = 12544 rows of 256 fp32 channels.
  - The per-row circular roll is done by the DMA engines via indirect
    (vector) DMA gather: one 512-elem (2048B) descriptor per row at
    element offset m-256 always contains the whole row; the rolled row
    is chunk[256:512] with the wrapped tail chunk[0:256] patched in
    where j >= 256-m (one Act-engine copy + one DVE copy_predicated
    with a wrap mask precomputed for all rows at setup).
  - Per-row offsets are computed once on-device from the shift map.
  - Gathers round-robin across 4 SWDGE queues; T=2 rows per tile keeps
    SWDGE descriptor generation pipelined under the DMA transfers.

Row layout: row(u,p,t) = u*(128*T) + p*T + t so stores are contiguous
DMAs per partition. The source x is staged in DRAM with a 256-element
zero pad on both ends so every descriptor stays in bounds; padded
reads only land in masked-out positions.
"""

import numpy as np

B, H, W, C = 32, 56, 56, 256
NCORES = 8
P = 128
RC = (B // NCORES) * H * W  # rows per core = 12544
COLS = RC // P  # 98 row-columns per partition
T = 2  # rows per partition per super-tile (kept <= Pool exec-queue depth
#        so SWDGE descriptor generation pipelines under the DMA transfers)
S = COLS // T  # 49 super-tiles
HALF = C // 2  # 128
PAD = C  # zero padding (elements) on each side of the flat source
NPAD = PAD + RC * C + PAD


def _setup(tc, cpool, shift_ap):
    """Load shifts; compute per-row gather offsets + patch thresholds."""
    import concourse.mybir as mybir

    nc = tc.nc
    i32 = mybir.dt.int32
    tt, ts = nc.vector.tensor_tensor, nc.vector.tensor_scalar

    m = cpool.tile([P, COLS], i32)
    nc.sync.dma_start(out=m[:], in_=shift_ap)

    # rowb[p, u*T+t] = PAD + 256 * (u*(P*T) + p*T + t)
    # (iota steps must fit int16, so emit the row index and scale on DVE)
    rowb = cpool.tile([P, COLS], i32)
    nc.gpsimd.iota(
        rowb[:],
        pattern=[[P * T, S], [1, T]],
        base=0,
        channel_multiplier=T,
    )
    ts(
        out=rowb[:],
        in0=rowb[:],
        scalar1=C,
        scalar2=PAD,
        op0=mybir.AluOpType.mult,
        op1=mybir.AluOpType.add,
    )

    scratch = cpool.tile([P, COLS], i32)
    # offC = rowb + m - 256: a 512-elem chunk from offC always contains the
    # whole row; out[j] = chunk[j+256] if j < 256-m else chunk[j].
    offC = cpool.tile([P, COLS], i32)
    tt(out=scratch[:], in0=rowb[:], in1=m[:], op=mybir.AluOpType.add)
    ts(out=offC[:], in0=scratch[:], scalar1=-C, scalar2=None, op0=mybir.AluOpType.add)
    # wrap mask, precomputed for all rows: wm[p, col, j] = (j >= 256 - m)
    j256 = cpool.tile([P, C], i32)
    nc.gpsimd.iota(j256[:], pattern=[[1, C]], base=0, channel_multiplier=0)
    m255 = cpool.tile([P, COLS], i32)
    ts(
        out=m255[:],
        in0=m[:],
        scalar1=-1,
        scalar2=C - 1,
        op0=mybir.AluOpType.mult,
        op1=mybir.AluOpType.add,
    )
    wm = cpool.tile([P, COLS, C], mybir.dt.int8)
    tt(
        out=wm[:],
        in0=j256[:].unsqueeze(1).to_broadcast([P, COLS, C]),
        in1=m255[:].to_broadcast([P, COLS, C]),
        op=mybir.AluOpType.is_gt,
    )
    return {"offC": offC, "wm": wm}


def _super_tile(tc, pool, consts, out_v, x_flat, u):
    """Gather, patch, store one super-tile (128 partitions x T rows).

    Per row: one 512-elem (2048B) descriptor starting at element m-256
    always contains the whole row; the rolled row is chunk[256:512] with
    the wrapped tail chunk[0:256] patched in where j >= 256-m.

    Alternate gather calls go on separate SWDGE queues so descriptor
    generation/processing for consecutive calls can proceed in parallel.
    """
    import concourse.bass as bass
    import concourse.mybir as mybir

    nc = tc.nc
    f32 = mybir.dt.float32
    csl = slice(u * T, (u + 1) * T)

    ch = pool.tile([P, T, 2 * C], f32)
    chv = ch[:].rearrange("p t c -> p (t c)")
    for t in range(T):
        col = u * T + t
        call = nc.gpsimd.indirect_dma_start(
            out=chv[:, t * 2 * C : (t + 1) * 2 * C],
            out_offset=None,
            in_=x_flat,
            in_offset=bass.IndirectOffsetOnAxis(
                ap=consts["offC"][:, col : col + 1], axis=1
            ),
        )
        q = (u * T + t) % 4
        if q:
            call.ins.queue = f"qPoolDynamic{q}"
    o = pool.tile([P, T, C], f32)
    nc.scalar.activation(
        out=o[:], in_=ch[:, :, C : 2 * C], func=mybir.ActivationFunctionType.Copy
    )
    nc.vector.copy_predicated(
        out=o[:], mask=consts["wm"][:, csl, :], data=ch[:, :, 0:C]
    )
    nc.sync.dma_start(out=out_v[:, u, :], in_=o[:].rearrange("p t c -> p (t c)"))


def _build(tc, out_ap, x_ap, shift_ap):
    """Emit the whole kernel body (setup + all super-tiles)."""
    out_v = out_ap.rearrange("(s p t) c -> p s (t c)", s=S, p=P, t=T)
    with tc.tile_pool(name="const", bufs=1) as cpool:
        consts = _setup(tc, cpool, shift_ap)
        with tc.tile_pool(name="work", bufs=16) as pool:
            for u in range(S):
                _super_tile(tc, pool, consts, out_v, x_ap, u)


def _shard_inputs(x, shift_map):
    """Full inputs -> per-core (x_pad [NPAD] f32, shift_perm [P, COLS] i32)."""
    x = np.ascontiguousarray(np.asarray(x), dtype=np.float32)
    sm = np.asarray(shift_map).astype(np.int32)
    bpc = B // NCORES
    in_maps = []
    for k in range(NCORES):
        xf = np.zeros((1, NPAD), np.float32)
        xf[0, PAD : PAD + RC * C] = x[k * bpc : (k + 1) * bpc].reshape(-1)
        sk = sm[k * bpc : (k + 1) * bpc].reshape(RC)
        # [p, u*T+t] = m of row u*(P*T) + p*T + t
        sperm = np.ascontiguousarray(
            sk.reshape(S, P, T).transpose(1, 0, 2).reshape(P, COLS)
        )
        in_maps.append({"x": xf, "shift_perm": sperm})
    return in_maps


_CACHE = {}


def _get_nc(repeat=1):
    key = ("nc", repeat)
    if key in _CACHE:
        return _CACHE[key]
    import concourse.mybir as mybir
    import concourse.tile as tile
    from concourse import bacc

    nc = bacc.Bacc(
        "TRN2",
        debug=False,
        enable_asserts=False,
        num_devices=NCORES,
        num_swdge_queues=4,
    )
    x_d = nc.dram_tensor("x", [1, NPAD], mybir.dt.float32, kind="ExternalInput")
    s_d = nc.dram_tensor("shift_perm", [P, COLS], mybir.dt.int32, kind="ExternalInput")
    o_d = nc.dram_tensor("out", [RC, C], mybir.dt.float32, kind="ExternalOutput")
    with tile.TileContext(nc) as tc:
        for _ in range(repeat):
            _build(tc, o_d.ap(), x_d.ap(), s_d.ap())
    nc.compile()
    _CACHE[key] = nc
    return nc


def kernel(x, shift_map, trace=False):
    from concourse.bass_utils import run_bass_kernel_spmd

    nc = _get_nc()
    in_maps = _shard_inputs(x, shift_map)
    res = run_bass_kernel_spmd(
        nc, in_maps, core_ids=list(range(NCORES)), trace=trace
    )
    bpc = B // NCORES
    out = np.concatenate(
        [r["out"].reshape(bpc, H, W, C) for r in res.results], axis=0
    )
    if trace:
        kernel.last_results = res
    return out
